# revision 18
# baseline (speedup 1.0000x reference)
"""MoE ExpertRouter kernel for 8x TRN2 NeuronCores (Bass/Tile).

Problem (hardcoded):
  x [8192, 1024] fp32; gate = softmax(relu(x@Wg1+bg1)@Wg2+bg2)  [8192, 8]
  h_e = relu(x@W1[e]+b1[e]); out_e = h_e@W2[e]+b2[e]
  out = sum_e gate[:, e] * out_e   [8192, 1024]

Strategy: data-parallel over tokens. Each of the 8 cores gets 1024 tokens
and computes the gate + all 8 experts for them; host concatenates the
per-core outputs. No collectives.

Per-core kernel layout:
  - host passes xT = x_shard.T [D=1024, T=1024] so the contraction dim (D)
    lands on SBUF partitions with no on-device transposes.
  - layer 1 (per expert, h-chunk of 1024): hT[h, tok] = relu(W1e.T-block
    matmuls vs xT) with per-partition bias via ScalarE activation.
  - layer 2: out[tok, dout] psum accumulation over the h-chunk k-tiles
    (lhsT = hT block, rhs = W2e rows); expert bias b2 folded in as a K=1
    matmul with a ones vector; gate-weighted accumulation into an SBUF
    fp32 accumulator via DVE scalar_tensor_tensor (out += gate_e * psum).
  - all matmuls use float32r (fp32 bits, FP22 multiply): full PE rate at
    free-dim >= 256, ~1e-4 relative error.
"""

import os

import numpy as np

import concourse.bass as bass
import concourse.mybir as mybir
import concourse.tile as tile
from concourse import bacc
from concourse.bass_utils import run_bass_kernel_spmd

F32 = mybir.dt.float32
F32R = mybir.dt.float32r

D = 1024          # input dim
H = 4096          # expert hidden dim
E = 8             # num experts
N_CORES = 8
N_TOKENS = 8192
P = 128           # SBUF partitions
HK = H // P       # 32 h k-tiles
DK = D // P       # 8 d k-tiles
HC = 8            # h k-tiles per chunk
NCHUNK = HK // HC  # 4 chunks
DOUT_N = 512      # layer-2 / layer-1 moving free dim


def _ceil_div(a, b):
    return (a + b - 1) // b


def build_nc(T):
    """Build the single-core Bass program for T tokens."""
    TM = T // P                     # token m-tiles
    tok_slices = []                 # (start, size) moving slices of tokens
    t0 = 0
    while t0 < T:
        sz = min(512, T - t0)
        tok_slices.append((t0, sz))
        t0 += sz

    nc = bacc.Bacc(
        "TRN2", target_bir_lowering=False, debug=False, num_devices=N_CORES
    )
    # Matmul-feeding tensors are float32r end-to-end (same fp32 bits; the
    # PE truncates to FP22 on read) so the walrus verifier sees a rounded
    # producer chain.
    xT = nc.dram_tensor("xT", [D, T], F32R, kind="ExternalInput").ap()
    Wg1 = nc.dram_tensor("Wg1", [D, H], F32R, kind="ExternalInput").ap()
    # host-prearranged biases/small weights (see kernel()):
    bg1T = nc.dram_tensor("bg1T", [P, HK], F32, kind="ExternalInput").ap()
    Wg2T = nc.dram_tensor("Wg2T", [P, HK, E], F32R, kind="ExternalInput").ap()
    bg2r = nc.dram_tensor("bg2r", [1, E], F32R, kind="ExternalInput").ap()
    W1 = nc.dram_tensor("W1", [E, D, H], F32R, kind="ExternalInput").ap()
    b1T = nc.dram_tensor("b1T", [P, E, HK], F32, kind="ExternalInput").ap()
    W2 = nc.dram_tensor("W2", [E, H, D], F32R, kind="ExternalInput").ap()
    b2r = nc.dram_tensor("b2r", [1, E * D], F32R, kind="ExternalInput").ap()
    out = nc.dram_tensor("out", [T, D], F32, kind="ExternalOutput").ap()

    with tile.TileContext(nc) as tc:
        _build(nc, tc, T, TM, tok_slices,
               xT, Wg1, bg1T, Wg2T, bg2r, W1, b1T, W2, b2r, out)
    nc.compile()
    return nc


def _build(nc, tc, T, TM, tok_slices,
           xT, Wg1, bg1T, Wg2T, bg2r, W1, b1T, W2, b2r, out):
    ctxs = []

    def pool(name, bufs, space="SBUF"):
        p = tc.tile_pool(name=name, bufs=bufs, space=space)
        ctxs.append(p)
        return p.__enter__()

    persist = pool("persist", 1)
    w1pool = pool("w1pool", 9)
    w2pool = pool("w2pool", 9)
    bpool = pool("bpool", 2)
    psum1 = pool("psum1", 2, space="PSUM")
    psum2 = pool("psum2", 3, space="PSUM")
    psumL = pool("psumL", 2, space="PSUM")
    small = pool("small", 4)

    # ---- persistent SBUF tensors ----
    xT_sb = persist.tile([P, DK, T], F32R, tag="xT_sb")
    nc.sync.dma_start(out=xT_sb[:], in_=xT.rearrange("(k p) t -> p k t", p=P))
    hT = persist.tile([P, HC, T], F32R, tag="hT")
    out_acc = persist.tile([P, TM, D], F32, tag="out_acc")
    gate_sb = persist.tile([P, TM * E], F32, tag="gate_sb")
    logits_sb = persist.tile([P, TM * E], F32, tag="logits_sb")
    bg1_sb = persist.tile([P, HK], F32, tag="bg1_sb")
    nc.sync.dma_start(out=bg1_sb[:], in_=bg1T[:, :])
    wg2_sb = persist.tile([P, HK, E], F32R, tag="wg2_sb")
    nc.sync.dma_start(out=wg2_sb[:], in_=Wg2T[:, :, :])
    bg2_sb = persist.tile([1, E], F32R, tag="bg2_sb")
    nc.sync.dma_start(out=bg2_sb[:], in_=bg2r[:, :])
    b1_sb = persist.tile([P, E, HK], F32, tag="b1_sb")
    nc.sync.dma_start(out=b1_sb[:], in_=b1T[:, :, :])
    ones_f = persist.tile([1, P], F32, tag="ones_f")
    nc.vector.memset(ones_f[:], 1.0)
    ones_sb = persist.tile([1, P], F32R, tag="ones_sb")
    nc.scalar.copy(ones_sb[:], ones_f[:])
    nc.vector.memset(out_acc[:], 0.0)

    def mm(ps, lhsT, rhs, start, stop):
        nc.tensor.matmul(ps, lhsT, rhs, start=start, stop=stop)

    def layer1(wtiles, bias_col, c):
        """hT[:, hm, :] = relu(sum_dk wtiles[dk][:,hm-block].T @ xT + bias)"""
        for hm in range(HC):
            for (ts, tsz) in tok_slices:
                ps = psum1.tile([P, DOUT_N], F32, tag="ps1")
                for dk in range(DK):
                    mm(ps[:, :tsz],
                       wtiles[dk][:, hm * P:(hm + 1) * P],
                       xT_sb[:, dk, ts:ts + tsz],
                       start=(dk == 0), stop=(dk == DK - 1))
                nc.scalar.activation(
                    hT[:, hm, ts:ts + tsz], ps[:, :tsz],
                    mybir.ActivationFunctionType.Relu,
                    bias=bias_col(hm) if callable(bias_col) else bias_col,
                )

    # ================= gate =================
    for c in range(NCHUNK):
        wtiles = []
        for dk in range(DK):
            t = w1pool.tile([P, HC * P], F32R, tag="w1t")
            nc.sync.dma_start(
                out=t[:], in_=Wg1[dk * P:(dk + 1) * P, c * H // NCHUNK:(c + 1) * H // NCHUNK])
            wtiles.append(t)
        layer1(wtiles, lambda hm, c=c: bg1_sb[:, c * HC + hm:c * HC + hm + 1], c)
        # logits partial: [tok, E] += hT_chunk.T-blocks @ Wg2 rows
        for m in range(TM):
            psL = psumL.tile([P, E], F32, tag="psL")
            for k in range(HC):
                last = (k == HC - 1) and (c != 0)
                mm(psL[:, :],
                   hT[:, k, m * P:(m + 1) * P],
                   wg2_sb[:, c * HC + k, :],
                   start=(k == 0), stop=last)
            if c == 0:
                # fold bg2 in once: += ones.T @ bg2 (K=1)
                mm(psL[:, :], ones_sb[:, :], bg2_sb[:, :], start=False, stop=True)
                nc.vector.tensor_copy(logits_sb[:, m * E:(m + 1) * E], psL[:, :])
            else:
                nc.vector.tensor_tensor(
                    out=logits_sb[:, m * E:(m + 1) * E],
                    in0=logits_sb[:, m * E:(m + 1) * E],
                    in1=psL[:, :], op=mybir.AluOpType.add)

    # softmax over E per token
    for m in range(TM):
        sl = logits_sb[:, m * E:(m + 1) * E]
        mx = small.tile([P, 1], F32, tag="mx")
        nc.vector.tensor_reduce(mx[:], sl, axis=mybir.AxisListType.X,
                                op=mybir.AluOpType.max)
        ex = small.tile([P, E], F32, tag="ex")
        nc.vector.tensor_scalar_sub(ex[:], sl, mx[:])
        nc.scalar.activation(ex[:], ex[:], mybir.ActivationFunctionType.Exp)
        sm = small.tile([P, 1], F32, tag="sm")
        nc.vector.tensor_reduce(sm[:], ex[:], axis=mybir.AxisListType.X,
                                op=mybir.AluOpType.add)
        rc = small.tile([P, 1], F32, tag="rc")
        nc.vector.reciprocal(rc[:], sm[:])
        nc.vector.tensor_scalar_mul(gate_sb[:, m * E:(m + 1) * E], ex[:], rc[:])

    # ================= experts =================
    for e in range(E):
        b2t = bpool.tile([1, D], F32R, tag="b2t")
        nc.sync.dma_start(out=b2t[:], in_=b2r[:, e * D:(e + 1) * D])
        for c in range(NCHUNK):
            w1tiles = []
            w2tiles = []
            for dk in range(DK):
                t = w1pool.tile([P, HC * P], F32R, tag="w1t")
                nc.sync.dma_start(
                    out=t[:],
                    in_=W1[e, dk * P:(dk + 1) * P,
                           c * H // NCHUNK:(c + 1) * H // NCHUNK])
                w1tiles.append(t)
            for k in range(HC):
                t = w2pool.tile([P, D], F32R, tag="w2t")
                nc.sync.dma_start(
                    out=t[:], in_=W2[e, (c * HC + k) * P:(c * HC + k + 1) * P, :])
                w2tiles.append(t)

            layer1(w1tiles,
                   lambda hm, e=e, c=c: b1_sb[:, e, c * HC + hm:c * HC + hm + 1],
                   c)

            # layer 2: accumulate over the chunk's h k-tiles
            for m in range(TM):
                for n in range(D // DOUT_N):
                    ps = psum2.tile([P, DOUT_N], F32, tag="ps2")
                    for k in range(HC):
                        last = (k == HC - 1) and (c != 0)
                        mm(ps[:, :],
                           hT[:, k, m * P:(m + 1) * P],
                           w2tiles[k][:, n * DOUT_N:(n + 1) * DOUT_N],
                           start=(k == 0), stop=last)
                    if c == 0:
                        # fold b2[e] in once per (m, n): += ones.T @ b2[e] (K=1)
                        mm(ps[:, :], ones_sb[:, :],
                           b2t[:, n * DOUT_N:(n + 1) * DOUT_N],
                           start=False, stop=True)
                    g = gate_sb[:, m * E + e:m * E + e + 1]
                    nc.vector.scalar_tensor_tensor(
                        out=out_acc[:, m, n * DOUT_N:(n + 1) * DOUT_N],
                        in0=ps[:, :], scalar=g,
                        in1=out_acc[:, m, n * DOUT_N:(n + 1) * DOUT_N],
                        op0=mybir.AluOpType.mult,
                        op1=mybir.AluOpType.add)

    # ================= store =================
    nc.sync.dma_start(out=out.rearrange("(m p) d -> p m d", p=P),
                      in_=out_acc[:])

    for p in reversed(ctxs):
        p.__exit__(None, None, None)


# ---------------- host side ----------------

_NC_CACHE = {}
LAST_RESULTS = None


def _get_nc(T):
    if T not in _NC_CACHE:
        _NC_CACHE[T] = build_nc(T)
    return _NC_CACHE[T]


def _prep_shared(Wg1, bg1, Wg2, bg2, W1, b1, W2, b2):
    """Host-side rearrangements shared by all cores."""
    Wg1 = np.ascontiguousarray(np.asarray(Wg1, dtype=np.float32))
    bg1 = np.asarray(bg1, dtype=np.float32)
    Wg2 = np.asarray(Wg2, dtype=np.float32)
    bg2 = np.asarray(bg2, dtype=np.float32)
    W1 = np.ascontiguousarray(np.asarray(W1, dtype=np.float32))
    b1 = np.asarray(b1, dtype=np.float32)
    W2 = np.ascontiguousarray(np.asarray(W2, dtype=np.float32))
    b2 = np.asarray(b2, dtype=np.float32)

    bg1T = np.ascontiguousarray(bg1.reshape(HK, P).T)                 # [128, 32]
    Wg2T = np.ascontiguousarray(Wg2.reshape(HK, P, E).transpose(1, 0, 2))  # [128,32,8]
    bg2r = np.ascontiguousarray(bg2.reshape(1, E))
    b1T = np.ascontiguousarray(b1.reshape(E, HK, P).transpose(2, 0, 1))    # [128,8,32]
    b2r = np.ascontiguousarray(b2.reshape(1, E * D))
    return dict(Wg1=Wg1, bg1T=bg1T, Wg2T=Wg2T, bg2r=bg2r,
                W1=W1, b1T=b1T, W2=W2, b2r=b2r)


def kernel(x, Wg1, bg1, Wg2, bg2, W1, b1, W2, b2):
    global LAST_RESULTS
    x = np.asarray(x, dtype=np.float32)
    n_tok = x.shape[0]
    T = n_tok // N_CORES
    nc = _get_nc(T)
    shared = _prep_shared(Wg1, bg1, Wg2, bg2, W1, b1, W2, b2)

    in_maps = []
    for i in range(N_CORES):
        xi = x[i * T:(i + 1) * T]
        m = dict(shared)
        m["xT"] = np.ascontiguousarray(xi.T)
        in_maps.append(m)

    trace = os.environ.get("BASS_KERNEL_TRACE", "0") == "1"
    tmpdir = os.environ.get("BASS_KERNEL_TRACE_DIR") if trace else None
    res = run_bass_kernel_spmd(nc, in_maps, list(range(N_CORES)), trace=trace,
                               tmpdir=tmpdir)
    LAST_RESULTS = res
    outs = [res.results[i]["out"] for i in range(N_CORES)]
    return np.concatenate(outs, axis=0).astype(np.float32)


# revision 34
# speedup vs baseline: 1.1033x; 1.1033x over previous
"""MoE ExpertRouter kernel for 8x TRN2 NeuronCores (Bass/Tile).

Problem (hardcoded):
  x [8192, 1024] fp32; gate = softmax(relu(x@Wg1+bg1)@Wg2+bg2)  [8192, 8]
  h_e = relu(x@W1[e]+b1[e]); out_e = h_e@W2[e]+b2[e]
  out = sum_e gate[:, e] * out_e   [8192, 1024]

Strategy: data-parallel over tokens. Each of the 8 cores gets 1024 tokens
and computes the gate + all 8 experts for them; host concatenates the
per-core outputs. No collectives.

Per-core kernel layout:
  - host passes xT = x_shard.T [D=1024, T=1024] so the contraction dim (D)
    lands on SBUF partitions with no on-device transposes.
  - layer 1 (per expert, h-chunk of 1024): hT[h, tok] = relu(W1e.T-block
    matmuls vs xT) with per-partition bias via ScalarE activation.
  - layer 2: out[tok, dout] psum accumulation over the h-chunk k-tiles
    (lhsT = hT block, rhs = W2e rows); expert bias b2 folded in as a K=1
    matmul with a ones vector; gate-weighted accumulation into an SBUF
    fp32 accumulator via DVE scalar_tensor_tensor (out += gate_e * psum).
  - all matmuls use float32r (fp32 bits, FP22 multiply): full PE rate at
    free-dim >= 256, ~1e-4 relative error.
"""

import os

import numpy as np

import concourse.bass as bass
import concourse.mybir as mybir
import concourse.tile as tile
from concourse import bacc
from concourse.bass_utils import run_bass_kernel_spmd

F32 = mybir.dt.float32
F32R = mybir.dt.float32r
F16 = mybir.dt.float16
# All dense-matmul operands in fp16 (walrus rejects mixed 16/32-bit pairs):
# enables FWL so LDWEIGHTS fully hides behind the moving stream. PSUM
# accumulation stays fp32. fp16 rounding is 2^-11 -> ~5e-4 rel err vs the
# fp32 reference (vs ~2e-4 for the all-float32r variant).
STATIONARY_F16 = os.environ.get("KERNEL_STATIONARY_F16", "1") == "1"
SDT = F16 if STATIONARY_F16 else F32R

D = 1024          # input dim
H = 4096          # expert hidden dim
E = 8             # num experts
N_CORES = 8
N_TOKENS = 8192
P = 128           # SBUF partitions
HK = H // P       # 32 h k-tiles
DK = D // P       # 8 d k-tiles
HC = 8            # h k-tiles per chunk
NCHUNK = HK // HC  # 4 chunks
DOUT_N = 512      # layer-2 / layer-1 moving free dim


def _ceil_div(a, b):
    return (a + b - 1) // b


def build_nc(T):
    """Build the single-core Bass program for T tokens."""
    TM = T // P                     # token m-tiles
    tok_slices = []                 # (start, size) moving slices of tokens
    t0 = 0
    while t0 < T:
        sz = min(512, T - t0)
        tok_slices.append((t0, sz))
        t0 += sz

    nc = bacc.Bacc(
        "TRN2", target_bir_lowering=False, debug=False, num_devices=N_CORES
    )
    # Matmul-feeding tensors are float32r end-to-end (same fp32 bits; the
    # PE truncates to FP22 on read) so the walrus verifier sees a rounded
    # producer chain.
    xT = nc.dram_tensor("xT", [D, T], SDT, kind="ExternalInput").ap()
    Wg1 = nc.dram_tensor("Wg1", [D, H], SDT, kind="ExternalInput").ap()
    # host-prearranged biases/small weights (see kernel()):
    bg1T = nc.dram_tensor("bg1T", [P, HK], F32, kind="ExternalInput").ap()
    Wg2T = nc.dram_tensor("Wg2T", [P, HK, E], SDT, kind="ExternalInput").ap()
    bg2r = nc.dram_tensor("bg2r", [1, E], F32R, kind="ExternalInput").ap()
    W1 = nc.dram_tensor("W1", [E, D, H], SDT, kind="ExternalInput").ap()
    b1T = nc.dram_tensor("b1T", [P, E, HK], F32, kind="ExternalInput").ap()
    W2 = nc.dram_tensor("W2", [E, H, D], SDT, kind="ExternalInput").ap()
    b2r = nc.dram_tensor("b2r", [1, E * D], F32R, kind="ExternalInput").ap()
    out = nc.dram_tensor("out", [T, D], F32, kind="ExternalOutput").ap()

    with tile.TileContext(nc) as tc:
        _build(nc, tc, T, TM, tok_slices,
               xT, Wg1, bg1T, Wg2T, bg2r, W1, b1T, W2, b2r, out)
    nc.compile()
    return nc


def _build(nc, tc, T, TM, tok_slices,
           xT, Wg1, bg1T, Wg2T, bg2r, W1, b1T, W2, b2r, out):
    ctxs = []

    def pool(name, bufs, space="SBUF"):
        p = tc.tile_pool(name=name, bufs=bufs, space=space)
        ctxs.append(p)
        return p.__enter__()

    persist = pool("persist", 1)
    w1pool = pool("w1pool", 9)
    w2pool = pool("w2pool", 9)
    bpool = pool("bpool", 2)
    psum1 = pool("psum1", 2, space="PSUM")
    psum2 = pool("psum2", 3, space="PSUM")
    psumL = pool("psumL", 2, space="PSUM")
    small = pool("small", 4)

    # ---- persistent SBUF tensors ----
    xT_sb = persist.tile([P, DK, T], SDT, tag="xT_sb")
    nc.sync.dma_start(out=xT_sb[:], in_=xT.rearrange("(k p) t -> p k t", p=P))
    hT = persist.tile([P, HC, T], SDT, tag="hT")
    out_acc = persist.tile([P, TM, D], F32, tag="out_acc")
    gate_sb = persist.tile([P, TM * E], F32, tag="gate_sb")
    logits_sb = persist.tile([P, TM * E], F32, tag="logits_sb")
    bg1_sb = persist.tile([P, HK], F32, tag="bg1_sb")
    nc.sync.dma_start(out=bg1_sb[:], in_=bg1T[:, :])
    wg2_sb = persist.tile([P, HK, E], SDT, tag="wg2_sb")
    nc.sync.dma_start(out=wg2_sb[:], in_=Wg2T[:, :, :])
    bg2_sb = persist.tile([1, E], F32R, tag="bg2_sb")
    nc.sync.dma_start(out=bg2_sb[:], in_=bg2r[:, :])
    b1_sb = persist.tile([P, E, HK], F32, tag="b1_sb")
    nc.sync.dma_start(out=b1_sb[:], in_=b1T[:, :, :])
    ones_f = persist.tile([1, P], F32, tag="ones_f")
    nc.vector.memset(ones_f[:], 1.0)
    ones_sb = persist.tile([1, P], F32R, tag="ones_sb")
    nc.scalar.copy(ones_sb[:], ones_f[:])
    nc.vector.memset(out_acc[:], 0.0)

    def mm(ps, lhsT, rhs, start, stop):
        nc.tensor.matmul(ps, lhsT, rhs, start=start, stop=stop)

    def layer1(wtiles, bias_col, c):
        """hT[:, hm, :] = relu(sum_dk wtiles[dk][:,hm-block].T @ xT + bias)"""
        for hm in range(HC):
            for (ts, tsz) in tok_slices:
                ps = psum1.tile([P, DOUT_N], F32, tag="ps1")
                for dk in range(DK):
                    mm(ps[:, :tsz],
                       wtiles[dk][:, hm * P:(hm + 1) * P],
                       xT_sb[:, dk, ts:ts + tsz],
                       start=(dk == 0), stop=(dk == DK - 1))
                nc.scalar.activation(
                    hT[:, hm, ts:ts + tsz], ps[:, :tsz],
                    mybir.ActivationFunctionType.Relu,
                    bias=bias_col(hm) if callable(bias_col) else bias_col,
                )

    # ================= gate =================
    for c in range(NCHUNK):
        wtiles = []
        for dk in range(DK):
            t = w1pool.tile([P, HC * P], SDT, tag="w1t")
            nc.sync.dma_start(
                out=t[:], in_=Wg1[dk * P:(dk + 1) * P, c * H // NCHUNK:(c + 1) * H // NCHUNK])
            wtiles.append(t)
        layer1(wtiles, lambda hm, c=c: bg1_sb[:, c * HC + hm:c * HC + hm + 1], c)
        # logits partial: [tok, E] += hT_chunk.T-blocks @ Wg2 rows
        for m in range(TM):
            psL = psumL.tile([P, E], F32, tag="psL")
            for k in range(HC):
                last = (k == HC - 1) and (c != 0)
                mm(psL[:, :],
                   hT[:, k, m * P:(m + 1) * P],
                   wg2_sb[:, c * HC + k, :],
                   start=(k == 0), stop=last)
            if c == 0:
                # fold bg2 in once: += ones.T @ bg2 (K=1)
                mm(psL[:, :], ones_sb[:, :], bg2_sb[:, :], start=False, stop=True)
                nc.vector.tensor_copy(logits_sb[:, m * E:(m + 1) * E], psL[:, :])
            else:
                nc.vector.tensor_tensor(
                    out=logits_sb[:, m * E:(m + 1) * E],
                    in0=logits_sb[:, m * E:(m + 1) * E],
                    in1=psL[:, :], op=mybir.AluOpType.add)

    # softmax over E per token
    for m in range(TM):
        sl = logits_sb[:, m * E:(m + 1) * E]
        mx = small.tile([P, 1], F32, tag="mx")
        nc.vector.tensor_reduce(mx[:], sl, axis=mybir.AxisListType.X,
                                op=mybir.AluOpType.max)
        ex = small.tile([P, E], F32, tag="ex")
        nc.vector.tensor_scalar_sub(ex[:], sl, mx[:])
        nc.scalar.activation(ex[:], ex[:], mybir.ActivationFunctionType.Exp)
        sm = small.tile([P, 1], F32, tag="sm")
        nc.vector.tensor_reduce(sm[:], ex[:], axis=mybir.AxisListType.X,
                                op=mybir.AluOpType.add)
        rc = small.tile([P, 1], F32, tag="rc")
        nc.vector.reciprocal(rc[:], sm[:])
        nc.vector.tensor_scalar_mul(gate_sb[:, m * E:(m + 1) * E], ex[:], rc[:])

    # ================= experts =================
    for e in range(E):
        b2t = bpool.tile([1, D], F32R, tag="b2t")
        nc.sync.dma_start(out=b2t[:], in_=b2r[:, e * D:(e + 1) * D])
        for c in range(NCHUNK):
            w1tiles = []
            w2tiles = []
            for dk in range(DK):
                t = w1pool.tile([P, HC * P], SDT, tag="w1t")
                nc.sync.dma_start(
                    out=t[:],
                    in_=W1[e, dk * P:(dk + 1) * P,
                           c * H // NCHUNK:(c + 1) * H // NCHUNK])
                w1tiles.append(t)
            for k in range(HC):
                t = w2pool.tile([P, D], SDT, tag="w2t")
                nc.sync.dma_start(
                    out=t[:], in_=W2[e, (c * HC + k) * P:(c * HC + k + 1) * P, :])
                w2tiles.append(t)

            layer1(w1tiles,
                   lambda hm, e=e, c=c: b1_sb[:, e, c * HC + hm:c * HC + hm + 1],
                   c)

            # layer 2: accumulate over the chunk's h k-tiles
            for m in range(TM):
                for n in range(D // DOUT_N):
                    ps = psum2.tile([P, DOUT_N], F32, tag="ps2")
                    for k in range(HC):
                        last = (k == HC - 1) and (c != 0)
                        mm(ps[:, :],
                           hT[:, k, m * P:(m + 1) * P],
                           w2tiles[k][:, n * DOUT_N:(n + 1) * DOUT_N],
                           start=(k == 0), stop=last)
                    if c == 0:
                        # fold b2[e] in once per (m, n): += ones.T @ b2[e] (K=1)
                        mm(ps[:, :], ones_sb[:, :],
                           b2t[:, n * DOUT_N:(n + 1) * DOUT_N],
                           start=False, stop=True)
                    g = gate_sb[:, m * E + e:m * E + e + 1]
                    nc.vector.scalar_tensor_tensor(
                        out=out_acc[:, m, n * DOUT_N:(n + 1) * DOUT_N],
                        in0=ps[:, :], scalar=g,
                        in1=out_acc[:, m, n * DOUT_N:(n + 1) * DOUT_N],
                        op0=mybir.AluOpType.mult,
                        op1=mybir.AluOpType.add)

    # ================= store =================
    nc.sync.dma_start(out=out.rearrange("(m p) d -> p m d", p=P),
                      in_=out_acc[:])

    for p in reversed(ctxs):
        p.__exit__(None, None, None)


# ---------------- host side ----------------

_NC_CACHE = {}
LAST_RESULTS = None


def _get_nc(T):
    if T not in _NC_CACHE:
        _NC_CACHE[T] = build_nc(T)
    return _NC_CACHE[T]


def _prep_shared(Wg1, bg1, Wg2, bg2, W1, b1, W2, b2):
    """Host-side rearrangements shared by all cores."""
    wdt = np.float16 if STATIONARY_F16 else np.float32
    Wg1 = np.ascontiguousarray(np.asarray(Wg1).astype(wdt))
    bg1 = np.asarray(bg1, dtype=np.float32)
    Wg2 = np.asarray(Wg2).astype(wdt)
    bg2 = np.asarray(bg2, dtype=np.float32)
    W1 = np.ascontiguousarray(np.asarray(W1).astype(wdt))
    b1 = np.asarray(b1, dtype=np.float32)
    W2 = np.ascontiguousarray(np.asarray(W2).astype(wdt))
    b2 = np.asarray(b2, dtype=np.float32)

    bg1T = np.ascontiguousarray(bg1.reshape(HK, P).T)                 # [128, 32]
    Wg2T = np.ascontiguousarray(Wg2.reshape(HK, P, E).transpose(1, 0, 2))  # [128,32,8]
    bg2r = np.ascontiguousarray(bg2.reshape(1, E))
    b1T = np.ascontiguousarray(b1.reshape(E, HK, P).transpose(2, 0, 1))    # [128,8,32]
    b2r = np.ascontiguousarray(b2.reshape(1, E * D))
    return dict(Wg1=Wg1, bg1T=bg1T, Wg2T=Wg2T, bg2r=bg2r,
                W1=W1, b1T=b1T, W2=W2, b2r=b2r)


def kernel(x, Wg1, bg1, Wg2, bg2, W1, b1, W2, b2):
    global LAST_RESULTS
    x = np.asarray(x, dtype=np.float32)
    n_tok = x.shape[0]
    T = n_tok // N_CORES
    nc = _get_nc(T)
    shared = _prep_shared(Wg1, bg1, Wg2, bg2, W1, b1, W2, b2)

    xdt = np.float16 if STATIONARY_F16 else np.float32
    in_maps = []
    for i in range(N_CORES):
        xi = x[i * T:(i + 1) * T]
        m = dict(shared)
        m["xT"] = np.ascontiguousarray(xi.T.astype(xdt))
        in_maps.append(m)

    trace = os.environ.get("BASS_KERNEL_TRACE", "0") == "1"
    tmpdir = os.environ.get("BASS_KERNEL_TRACE_DIR") if trace else None
    res = run_bass_kernel_spmd(nc, in_maps, list(range(N_CORES)), trace=trace,
                               tmpdir=tmpdir)
    LAST_RESULTS = res
    outs = [res.results[i]["out"] for i in range(N_CORES)]
    return np.concatenate(outs, axis=0).astype(np.float32)
